# revision 1
# baseline (speedup 1.0000x reference)
"""Trainium2 Bass kernel for 16-head MultiHeadAttention (B=4, S=2048, D=1024).

Sharding: 8 cores = 4 batches x 2 head-groups (Megatron-style tensor
parallelism inside a batch).  Core c handles batch c//2 and heads
(c%2)*8 .. +8.  Q/K/V projection weights are column-sharded, Wo is
row-sharded; the 2-way partial sum of the output projection plus the bo
bias is applied on the host after gathering.

Device layout choices (per core):
  - Activations arrive host-pre-transposed: X^T [1024, 2048].
  - Q and K are produced directly in transposed layout QT/KT [d, s]
    (head dim on partitions), V in natural layout [s, d] with a ones
    column appended per head (so the attention row-sum rides along the
    ctx matmul as output row 64).
  - Attention runs in transposed orientation: logitsT [sk, sq] =
    (K Q^T), so softmax's additive mask is a per-partition ACT bias and
    exp needs no max-subtraction (logits are O(1) by construction).
  - ctx^T [d, sq] = V^T @ attnT accumulates over sk tiles in PSUM;
    row 64 is the softmax denominator.  Normalization multiplies by the
    broadcast reciprocal before the output projection.
"""

import os
import sys

for _p in ("/opt/trn_rl_repo", "/root/.axon_site/_ro/trn_rl_repo"):
    if os.path.isdir(_p) and _p not in sys.path:
        sys.path.insert(0, _p)

import numpy as np

import concourse.bass as bass
import concourse.mybir as mybir
import concourse.tile as tile
from concourse import bacc
from concourse.bass_utils import run_bass_kernel_spmd

# ---------------------------------------------------------------- constants
B = 4
S = 2048
D = 1024
NH = 16          # total heads
DK = 64          # head dim
N_CORES = 8
H = 8            # heads per core
DH = H * DK      # 512 local d_model columns
KT_N = D // 128  # 8 contraction k-tiles
NDT = DH // 128  # 4 d-tiles of QT/KT
NSC = S // 512   # 4 s-chunks
NST = S // 128   # 16 s-tiles
VSTRIDE = H * (DK + 1)  # 520: v tile columns per s-tile (ones col per head)
ECH = 512
NECH = D // ECH  # 2 output column chunks
SCALE = 1.0 / np.sqrt(np.float32(DK))

F32 = mybir.dt.float32
F32R = mybir.dt.float32r

BF16 = mybir.dt.bfloat16

# matmul input dtype mode: "f32" (exact, 4 cyc/row), "f32r" (1 cyc/row),
# or "bf16" (1 cyc/row, half SBUF/DMA, separate overlapped weight loads)
MM_MODE = os.environ.get("MHA_MM_MODE", "f32r")
MMDT = {"f32": F32, "f32r": F32R, "bf16": BF16}[MM_MODE]
NPDT = np.float32 if MM_MODE != "bf16" else None  # host array dtype, see prep
# 4-byte matmul dtypes double every resident tile, so shrink buffering to
# fit the 192KB/partition SBUF; bf16 affords deeper pipelines.
WIDE = MM_MODE != "bf16"

# replicate the body inside one NEFF (timing aid; grading uses 1)
ITERS = int(os.environ.get("MHA_ITERS", "1"))
# comma-separated phases to skip, for differential timing (never set when
# grading): exp, norm, ctx, oproj, attn
SKIP = set(filter(None, os.environ.get("MHA_SKIP", "").split(",")))
if "attn" in SKIP:
    SKIP |= {"lg", "exp", "ctx", "norm", "oproj"}
if "ctxdown" in SKIP:
    SKIP |= {"ctx", "norm", "oproj"}
if "compute" in SKIP:
    SKIP |= {"projmm", "lg", "exp", "ctx", "norm", "oproj"}

_CACHE = {}


def build_kernel(has_bias: bool):
    nc = bacc.Bacc(
        "TRN2",
        target_bir_lowering=False,
        debug=False,
        num_devices=N_CORES,
        dynamic_dma_scratch_size=2048,
    )

    KR = D + (1 if has_bias else 0)  # input rows incl. optional bias row
    xq = nc.dram_tensor("xq", (KR, S), MMDT, kind="ExternalInput")
    xk = nc.dram_tensor("xk", (KR, S), MMDT, kind="ExternalInput")
    xv = nc.dram_tensor("xv", (KR, S), MMDT, kind="ExternalInput")
    wq = nc.dram_tensor("wq", (KR, DH), MMDT, kind="ExternalInput")
    wk = nc.dram_tensor("wk", (KR, DH), MMDT, kind="ExternalInput")
    wv = nc.dram_tensor("wv", (KR, DH), MMDT, kind="ExternalInput")
    wo = nc.dram_tensor("wo", (DH, D), MMDT, kind="ExternalInput")
    mb = nc.dram_tensor("mb", (128, NST), F32, kind="ExternalInput")
    ones_d = nc.dram_tensor("ones", (128, NST * H), MMDT, kind="ExternalInput")
    out = nc.dram_tensor("out", (S, D), F32, kind="ExternalOutput")

    with tile.TileContext(nc) as tc:
        with (
            tc.tile_pool(name="persist", bufs=1) as pp,
            tc.tile_pool(name="xpool", bufs=1 if WIDE else 2) as xp,
            tc.tile_pool(name="wpool", bufs=1 if WIDE else 2) as wp,
            tc.tile_pool(name="augp", bufs=2) as ap_,
            tc.tile_pool(name="wop", bufs=1) as wop,
            tc.tile_pool(name="atp", bufs=3 if WIDE else 4) as atp,
            tc.tile_pool(name="nrm", bufs=1 if WIDE else 2) as nrm,
            tc.tile_pool(name="ctxn", bufs=1 if WIDE else 2) as cxp,
            tc.tile_pool(name="obp", bufs=1 if WIDE else 2) as obp,
            tc.tile_pool(name="psum", bufs=1, space="PSUM") as psp,
        ):
            qt_sb = pp.tile([128, NDT * S], MMDT, tag="qt")    # [d-tile | s]
            kt_sb = pp.tile([128, NDT * S], MMDT, tag="kt")
            v_sb = pp.tile([128, NST * VSTRIDE], MMDT, tag="v")
            mb_sb = pp.tile([128, NST], F32, tag="mb")
            wo_sb = wop.tile([64, H * D], MMDT, tag="wo")
            ones32 = pp.tile([65, 64], F32, tag="ones32")
            nc.vector.memset(ones32[:], 1.0)

            for _rep in range(ITERS):
                nc.sync.dma_start(mb_sb[:], mb[:, :])
                # ones columns for the rowsum trick, one strided DMA
                nc.sync.dma_start(
                    v_sb[:].rearrange("p (st c) -> p st c", c=VSTRIDE)
                        [:, :, DK:VSTRIDE:DK + 1],
                    ones_d[:].rearrange("p (st h) -> p st h", h=H),
                )
                # wo: one DMA, head-pair rows land as [64, h*D] blocks
                nc.sync.dma_start(
                    wo_sb[:].rearrange("p (h e) -> p h e", e=D),
                    wo[:].rearrange("(h p) e -> p h e", p=DK),
                )

                # ------------------------------------------ projections
                # order: v, k, q(j-outer) so attention unblocks early
                for which, xd, wd in (("v", xv, wv), ("k", xk, wk),
                                      ("q", xq, wq)):
                    w_sb = wp.tile([128, KT_N * DH], MMDT, tag="w",
                                   name=f"w_{which}")
                    nc.sync.dma_start(
                        w_sb[:].rearrange("p (kt d) -> p kt d", d=DH),
                        wd[0:D, :].rearrange("(kt p) d -> p kt d", p=128),
                    )
                    if has_bias:
                        w_aug = ap_.tile([1, DH], MMDT, tag="waug")
                        nc.sync.dma_start(w_aug[:], wd[D:D + 1, :])

                    for half in range(2):
                        s0 = half * 1024
                        x_sb = xp.tile([128, KT_N * 1024], MMDT, tag="x",
                                       name=f"x_{which}{half}")
                        nc.sync.dma_start(
                            x_sb[:].rearrange("p (kt s) -> p kt s", s=1024),
                            xd[0:D, s0:s0 + 1024].rearrange(
                                "(kt p) s -> p kt s", p=128),
                        )
                        if has_bias:
                            x_aug = ap_.tile([1, 1024], MMDT, tag="xaug",
                                             name=f"xa_{which}{half}")
                            nc.sync.dma_start(x_aug[:],
                                              xd[D:D + 1, s0:s0 + 1024])

                        if which in ("q", "k"):
                            dst = qt_sb if which == "q" else kt_sb
                            for j2 in range(2):
                                for dt in range(NDT):
                                    psqk = psp.tile([128, 512], F32,
                                                    tag="proj", bufs=2)
                                    if "projmm" not in SKIP:
                                        for kt in range(KT_N):
                                            nc.tensor.matmul(
                                                psqk[:],
                                                w_sb[:, kt * DH + dt * 128:
                                                     kt * DH + dt * 128 + 128],
                                                x_sb[:, kt * 1024 + j2 * 512:
                                                     kt * 1024 + j2 * 512 + 512],
                                                start=(kt == 0),
                                                stop=(kt == KT_N - 1
                                                      and not has_bias),
                                            )
                                        if has_bias:
                                            nc.tensor.matmul(
                                                psqk[:],
                                                w_aug[0:1,
                                                      dt * 128:dt * 128 + 128],
                                                x_aug[0:1,
                                                      j2 * 512:j2 * 512 + 512],
                                                start=False, stop=True,
                                            )
                                        nc.vector.tensor_copy(
                                            dst[:, dt * S + s0 + j2 * 512:
                                                dt * S + s0 + j2 * 512 + 512],
                                            psqk[:],
                                        )
                        else:  # V: natural layout per s-tile, per-head cols
                            for st4 in range(8):
                                st = half * 8 + st4
                                psv = psp.tile([128, 512], F32, tag="proj",
                                               bufs=2)
                                if "projmm" in SKIP:
                                    continue
                                for kt in range(KT_N):
                                    nc.tensor.matmul(
                                        psv[:],
                                        x_sb[:, kt * 1024 + st4 * 128:
                                             kt * 1024 + st4 * 128 + 128],
                                        w_sb[:, kt * DH:(kt + 1) * DH],
                                        start=(kt == 0),
                                        stop=(kt == KT_N - 1 and not has_bias),
                                    )
                                if has_bias:
                                    nc.tensor.matmul(
                                        psv[:],
                                        x_aug[0:1, st4 * 128:st4 * 128 + 128],
                                        w_aug[0:1, :],
                                        start=False, stop=True,
                                    )
                                # per-head copy into aug layout (ones cols
                                # at stride DK+1 stay untouched)
                                nc.vector.tensor_copy(
                                    v_sb[:, st * VSTRIDE:
                                         st * VSTRIDE + H * (DK + 1)]
                                    .rearrange("p (h c) -> p h c", c=DK + 1)
                                    [:, :, 0:DK],
                                    psv[:].rearrange("p (h c) -> p h c", c=DK),
                                )

                # --------------------------------- attention + out-proj
                for j in range(NSC):
                    ctxn = cxp.tile([64, H * 512], MMDT, tag="ctxn")
                    for hp in range(NDT):
                        heads = (2 * hp, 2 * hp + 1)
                        ctx_ps = {h: psp.tile([65, 512], F32, tag="ctxbc",
                                              bufs=3, name=f"ctx{h}")
                                  for h in heads}
                        for i in range(NST):
                            lgs = {}
                            for h in heads:  # adjacent emits: row groups pair
                                pb = (h % 2) * 64
                                lgs[h] = psp.tile([128, 512], F32, tag="big",
                                                  bufs=3, name=f"lg{h}")
                                if "lg" not in SKIP:
                                    nc.tensor.matmul(
                                        lgs[h][:],
                                        kt_sb[pb:pb + 64,
                                              hp * S + i * 128:
                                              hp * S + i * 128 + 128],
                                        qt_sb[pb:pb + 64,
                                              hp * S + j * 512:
                                              hp * S + j * 512 + 512],
                                        start=True, stop=True,
                                    )
                            for h in heads:
                                at = atp.tile([128, 512], MMDT, tag="at",
                                              name=f"at{h}")
                                if "exp" in SKIP:
                                    if "lg" not in SKIP:
                                        nc.vector.tensor_copy(at[:], lgs[h][:])
                                else:
                                    nc.scalar.activation(
                                        at[:], lgs[h][:],
                                        mybir.ActivationFunctionType.Exp,
                                        bias=mb_sb[:, i:i + 1],
                                        scale=float(SCALE),
                                    )
                                if "ctx" not in SKIP:
                                    nc.tensor.matmul(
                                        ctx_ps[h][:],
                                        v_sb[:, i * VSTRIDE + h * (DK + 1):
                                             i * VSTRIDE + h * (DK + 1) + DK + 1],
                                        at[:],
                                        start=(i == 0), stop=(i == NST - 1),
                                    )
                        for h in heads:
                            if "norm" in SKIP or "ctx" in SKIP:
                                if "ctx" not in SKIP:
                                    nc.vector.tensor_copy(
                                        ctxn[:, h * 512:(h + 1) * 512],
                                        ctx_ps[h][0:64, :])
                                continue
                            # reciprocal of the rowsum row, in place at
                            # partition 64; then outer-product broadcast
                            # via a row-64-aligned K=1 matmul (no DMA)
                            rsr = nrm.tile([65, 512], F32, tag="rsr",
                                           name=f"rsr{h}")
                            nc.vector.reciprocal(rsr[64:65, :],
                                                 ctx_ps[h][64:65, :])
                            bc = psp.tile([64, 512], F32, tag="ctxbc",
                                          bufs=3, name=f"bc{h}")
                            nc.tensor.matmul(
                                bc[:],
                                ones32[64:65, :],
                                rsr[64:65, :],
                                start=True, stop=True,
                            )
                            bc_sb = nrm.tile([64, 512], F32, tag="bcsb",
                                             name=f"bcsb{h}")
                            nc.vector.tensor_copy(bc_sb[:], bc[:])
                            nc.vector.tensor_mul(
                                ctxn[:, h * 512:(h + 1) * 512],
                                ctx_ps[h][0:64, :], bc_sb[:],
                            )
                    # output projection for s-chunk j
                    for t in range(4):
                        sq = j * 512 + t * 128
                        ob = obp.tile([128, D], F32, tag="ob")
                        for ec in range(NECH):
                            if "oproj" in SKIP:
                                continue
                            po = psp.tile([128, 512], F32, tag="big", bufs=3,
                                          name="po")
                            for h in range(H):
                                nc.tensor.matmul(
                                    po[:],
                                    ctxn[0:64, h * 512 + t * 128:
                                         h * 512 + t * 128 + 128],
                                    wo_sb[0:64, h * D + ec * ECH:
                                          h * D + ec * ECH + ECH],
                                    start=(h == 0), stop=(h == H - 1),
                                )
                            nc.vector.tensor_copy(
                                ob[:, ec * ECH:ec * ECH + ECH], po[:])
                        if "oproj" not in SKIP:
                            nc.sync.dma_start(out[sq:sq + 128, :], ob[:])

    nc.compile()
    return nc


def _get_kernel(has_bias: bool):
    key = (has_bias, MM_MODE)
    if key not in _CACHE:
        _CACHE[key] = build_kernel(has_bias)
    return _CACHE[key]


def prep_in_maps(query, key, value, mask, Wq, bq, Wk, bk, Wv, bv, Wo, bo):
    query = np.asarray(query, dtype=np.float32)
    key = np.asarray(key, dtype=np.float32)
    value = np.asarray(value, dtype=np.float32)
    mask = np.asarray(mask, dtype=np.float32)
    Wq = np.asarray(Wq, dtype=np.float32)
    Wk = np.asarray(Wk, dtype=np.float32)
    Wv = np.asarray(Wv, dtype=np.float32)
    Wo = np.asarray(Wo, dtype=np.float32)
    bq = np.asarray(bq, dtype=np.float32)
    bk = np.asarray(bk, dtype=np.float32)
    bv = np.asarray(bv, dtype=np.float32)
    bo = np.asarray(bo, dtype=np.float32)

    has_bias = bool(np.any(bq) or np.any(bk) or np.any(bv))

    ones_row = np.ones((1, S), dtype=np.float32)
    in_maps = []
    for c in range(N_CORES):
        b, g = divmod(c, 2)
        cols = slice(g * DH, (g + 1) * DH)
        xq_c = np.ascontiguousarray(query[b].T)
        xk_c = np.ascontiguousarray(key[b].T)
        xv_c = np.ascontiguousarray(value[b].T)
        wq_c = np.ascontiguousarray(Wq[:, cols])
        wk_c = np.ascontiguousarray(Wk[:, cols])
        wv_c = np.ascontiguousarray(Wv[:, cols])
        if has_bias:
            xq_c = np.concatenate([xq_c, ones_row], axis=0)
            xk_c = np.concatenate([xk_c, ones_row], axis=0)
            xv_c = np.concatenate([xv_c, ones_row], axis=0)
            wq_c = np.concatenate([wq_c, bq[None, cols]], axis=0)
            wk_c = np.concatenate([wk_c, bk[None, cols]], axis=0)
            wv_c = np.concatenate([wv_c, bv[None, cols]], axis=0)
        mb_c = np.ascontiguousarray(
            mask[b, 0, 0].reshape(NST, 128).T * np.float32(-1e9))
        in_maps.append({
            "xq": xq_c, "xk": xk_c, "xv": xv_c,
            "wq": wq_c, "wk": wk_c, "wv": wv_c,
            "wo": np.ascontiguousarray(Wo[cols, :]),
            "mb": mb_c,
            "ones": np.ones((128, NST * H), dtype=np.float32),
        })
    if MM_MODE == "bf16":
        import ml_dtypes
        for m in in_maps:
            for k in m:
                if k != "mb":
                    m[k] = m[k].astype(ml_dtypes.bfloat16)
    return has_bias, in_maps, bo


def assemble_out(per_core_out, bo):
    out = np.empty((B, S, D), dtype=np.float32)
    for b in range(B):
        out[b] = per_core_out[2 * b] + per_core_out[2 * b + 1] + bo
    return out


def kernel(**inputs):
    has_bias, in_maps, bo = prep_in_maps(**inputs)
    nc = _get_kernel(has_bias)
    res = run_bass_kernel_spmd(nc, in_maps, core_ids=list(range(N_CORES)))
    return assemble_out([res.results[c]["out"] for c in range(N_CORES)], bo)


if __name__ == "__main__":
    rng = np.random.default_rng(0)
    q = rng.standard_normal((B, S, D)).astype(np.float32)
    k = rng.standard_normal((B, S, D)).astype(np.float32)
    v = rng.standard_normal((B, S, D)).astype(np.float32)
    m = np.zeros((B, 1, 1, S), dtype=np.float32)
    sc = 1.0 / np.sqrt(D)
    Wq = (rng.standard_normal((D, D)) * sc).astype(np.float32)
    Wk = (rng.standard_normal((D, D)) * sc).astype(np.float32)
    Wv = (rng.standard_normal((D, D)) * sc).astype(np.float32)
    Wo = (rng.standard_normal((D, D)) * sc).astype(np.float32)
    z = np.zeros(D, dtype=np.float32)
    o = kernel(q, k, v, m, Wq, z, Wk, z, Wv, z, Wo, z)
    print("out", o.shape, o.dtype, float(np.abs(o).mean()))



# revision 36
# speedup vs baseline: 1.9198x; 1.9198x over previous
"""Trainium2 Bass kernel for 16-head MultiHeadAttention (B=4, S=2048, D=1024).

Sharding: 8 cores = 4 batches x 2 head-groups (Megatron-style tensor
parallelism inside a batch).  Core c handles batch c//2 and heads
(c%2)*8 .. +8.  Q/K/V projection weights are column-sharded, Wo is
row-sharded; the 2-way partial sum of the output projection plus the bo
bias is applied on the host after gathering.

Device layout (per core), engine-balanced around three facts:
  - ACT (exp) runs 1 elem/lane/cycle at 1.2 GHz with a ~222-cycle
    per-instruction access overhead, so exp is issued on [128, 1024]
    tiles spanning two k-tiles (two PSUM banks) at once.
  - PE attention matmuls are capped at 50% array utilization by dk=64,
    so attention PE time ~= exp ACT time; the two pipeline per 2-k-tile
    group: logits mm (PE) -> exp (ACT) -> ctx mm (PE).
  - The output projection contracts d_model, so head-pair-stacked
    normalized ctx tiles [128, 512q] serve as stationary operands with
    full K=128, halving out-proj PE time vs per-head K=64.

All matmul inputs are bf16 (same PE rate as f32r, half the DMA/SBUF);
PSUM accumulation stays f32.  Q and K are produced in transposed layout
[dk, s] (head dim on partitions), V in natural layout [s, d] with a
ones column per head so the softmax denominator rides along the ctx
matmul as output row 64.  The per-(head, q) reciprocal is broadcast
across 64 partitions with a K=1 f32r matmul.

Phase order per core: K proj, V proj, Q proj (chunk 0) upfront; then
per 512-wide q-chunk j: 8 heads of attention with the Q projection for
chunk j+1 interleaved between heads (keeps ACT fed while PE detours),
normalization into head-pair tiles, then the out-projection.
"""

import os
import sys

for _p in ("/opt/trn_rl_repo", "/root/.axon_site/_ro/trn_rl_repo"):
    if os.path.isdir(_p) and _p not in sys.path:
        sys.path.insert(0, _p)

import numpy as np

import concourse.bass as bass
import concourse.mybir as mybir
import concourse.tile as tile
from concourse import bacc
from concourse.bass_utils import run_bass_kernel_spmd

# ---------------------------------------------------------------- constants
B = 4
S = 2048
D = 1024
NH = 16          # total heads
DK = 64          # head dim
N_CORES = 8
H = 8            # heads per core
DH = H * DK      # 512 local d_model columns
KT_N = D // 128  # 8 contraction k-tiles
NDT = DH // 128  # 4 d-tiles of QT/KT
NSC = S // 512   # 4 s-chunks (q-chunks)
NST = S // 128   # 16 s-tiles (k-tiles)
VSTRIDE = H * (DK + 1)  # 520: v tile columns per s-tile (ones col per head)
SCALE = 1.0 / np.sqrt(np.float32(DK))
G = 2            # k-tiles per exp group
NG = NST // G    # 8 exp groups per (head, q-chunk)

F32 = mybir.dt.float32
F32R = mybir.dt.float32r
BF16 = mybir.dt.bfloat16

MM_MODE = os.environ.get("MHA_MM_MODE", "bf16")
PHASE_A = os.environ.get("MHA_PHASE_A", "1") == "1"
MMDT = {"f32": F32, "f32r": F32R, "bf16": BF16}[MM_MODE]

# replicate the body inside one NEFF (timing aid; grading uses 1)
ITERS = int(os.environ.get("MHA_ITERS", "1"))

_CACHE = {}


def build_kernel(has_bias: bool, has_mask: bool):
    nc = bacc.Bacc(
        "TRN2",
        target_bir_lowering=False,
        debug=False,
        num_devices=N_CORES,
        dynamic_dma_scratch_size=2048,
    )

    KR = D + (1 if has_bias else 0)  # input rows incl. optional bias row
    xq = nc.dram_tensor("xq", (KR, S), MMDT, kind="ExternalInput")
    xk = nc.dram_tensor("xk", (KR, S), MMDT, kind="ExternalInput")
    xv = nc.dram_tensor("xv", (KR, S), MMDT, kind="ExternalInput")
    wq = nc.dram_tensor("wq", (KR, DH), MMDT, kind="ExternalInput")
    wk = nc.dram_tensor("wk", (KR, DH), MMDT, kind="ExternalInput")
    wv = nc.dram_tensor("wv", (KR, DH), MMDT, kind="ExternalInput")
    wo = nc.dram_tensor("wo", (DH, D), MMDT, kind="ExternalInput")
    if has_mask:
        mb = nc.dram_tensor("mb", (128, NST), F32, kind="ExternalInput")
    out = nc.dram_tensor("out", (S, D), F32, kind="ExternalOutput")

    with tile.TileContext(nc) as tc:
        with (
            tc.tile_pool(name="persist", bufs=1) as pp,
            tc.tile_pool(name="xpool", bufs=6) as xp,
            tc.tile_pool(name="wpool", bufs=3) as wp,
            tc.tile_pool(name="augp", bufs=2) as ap_,
            tc.tile_pool(name="atp", bufs=18) as atp,
            tc.tile_pool(name="nrm", bufs=2) as nrm,
            tc.tile_pool(name="ctx2p", bufs=8) as cxp,
            tc.tile_pool(name="obp", bufs=2) as obp,
            tc.tile_pool(name="psum", bufs=1, space="PSUM") as psp,
        ):
            qt_sb = pp.tile([128, NDT * S], MMDT, tag="qt")    # [d-tile | s]
            kt_sb = pp.tile([128, NDT * S], MMDT, tag="kt")
            v_sb = pp.tile([128, NST * VSTRIDE], MMDT, tag="v")
            wo_sb = pp.tile([128, NDT * D], MMDT, tag="wo")    # [pair | e]
            ones1f = pp.tile([1, DK], F32, tag="ones1f")
            nc.vector.memset(ones1f[:], 1.0)
            ones1 = pp.tile([1, DK], F32R, tag="ones1")
            with nc.allow_low_precision("f32r shares f32 bits"):
                nc.vector.tensor_copy(ones1[:], ones1f[:])
            if has_mask:
                mb_sb = pp.tile([128, NST], F32, tag="mb")

            def proj_psum():
                return psp.tile([128, 512], F32, tag="sh", bufs=2, name="pp")

            def qk_quarter(x_sb, w_sb, w_aug, x_aug, dst, q, dts):
                """Emit Q/K projection matmuls for s-quarter q, d-tiles dts."""
                for dt in dts:
                    ps = proj_psum()
                    for kt in range(KT_N):
                        nc.tensor.matmul(
                            ps[:],
                            w_sb[:, kt * DH + dt * 128:kt * DH + dt * 128 + 128],
                            x_sb[:, kt * 512:kt * 512 + 512],
                            start=(kt == 0),
                            stop=(kt == KT_N - 1 and not has_bias),
                        )
                    if has_bias:
                        nc.tensor.matmul(
                            ps[:],
                            w_aug[0:1, dt * 128:dt * 128 + 128],
                            x_aug[0:1, :],
                            start=False, stop=True,
                        )
                    nc.vector.tensor_copy(
                        dst[:, dt * S + q * 512:dt * S + q * 512 + 512], ps[:])

            def load_w(wd, name, split=False, eng=None):
                eng = eng or nc.sync
                w_sb = wp.tile([128, KT_N * DH], MMDT, tag="w", name=name)
                halves = 2 if split else 1
                hk = KT_N // halves
                for hh in range(halves):
                    eng.dma_start(
                        w_sb[:].rearrange("p (kt d) -> p kt d", d=DH)
                            [:, hh * hk:(hh + 1) * hk, :],
                        wd[hh * hk * 128:(hh + 1) * hk * 128, :].rearrange(
                            "(kt p) d -> p kt d", p=128),
                    )
                w_aug = None
                if has_bias:
                    w_aug = ap_.tile([1, DH], MMDT, tag="waug", name=f"a{name}")
                    eng.dma_start(w_aug[:], wd[D:D + 1, :])
                return w_sb, w_aug

            def load_x(xd, q, name, split=False, eng=None):
                eng = eng or nc.sync
                x_sb = xp.tile([128, KT_N * 512], MMDT, tag="x", name=name)
                halves = 2 if split else 1
                hk = KT_N // halves
                for hh in range(halves):
                    eng.dma_start(
                        x_sb[:].rearrange("p (kt s) -> p kt s", s=512)
                            [:, hh * hk:(hh + 1) * hk, :],
                        xd[hh * hk * 128:(hh + 1) * hk * 128,
                           q * 512:q * 512 + 512].rearrange(
                            "(kt p) s -> p kt s", p=128),
                    )
                x_aug = None
                if has_bias:
                    x_aug = ap_.tile([1, 512], MMDT, tag="xaug", name=f"a{name}")
                    eng.dma_start(x_aug[:], xd[D:D + 1, q * 512:q * 512 + 512])
                return x_sb, x_aug

            for _rep in range(ITERS):
                # ------------------------------------------- phase A
                # K projection d-tile-major (needs all 4 x quarters live)
                # so heads 0-1 of chunk 0 can start logits+exp right after
                # K-dt0 + Q0-dt0, ~12us in, keeping ACT fed while the rest
                # of the prologue runs on the PE.  Their ctx matmuls are
                # deferred until the V projection delivers the s-tiles.
                w_sb, w_aug = load_w(wk, "w_k", split=True)
                xk0, xk0_aug = load_x(xk, 0, "x_k0", split=True)
                wq_sb, wq_aug = load_w(wq, "w_q")
                xq0, xq0_aug = load_x(xq, 0, "x_q0")
                xks = [(xk0, xk0_aug)]
                xks.append(load_x(xk, 1, f"x_k1", split=True))
                xks.append(load_x(xk, 2, f"x_k2", split=True))
                xks.append(load_x(xk, 3, f"x_k3", split=True))
                # prefetch V's first inputs so the V phase starts instantly
                wv_sb, wv_aug = load_w(wv, "w_v")
                xv0 = load_x(xv, 0, "x_v0")
                if has_mask:
                    nc.sync.dma_start(mb_sb[:], mb[:, :])
                # ones columns for the rowsum trick: memset everything to 1,
                # the V-proj copies then overwrite the data columns and the
                # per-head ones columns (stride DK+1) survive
                nc.vector.memset(v_sb[:], 1.0)

                # --------------------------------- attention + out-proj
                # Software-pipelined emission: PE stream per slot is
                # [logits(g), one spread matmul, ctx(g-1)] so the in-order
                # PE never waits on the exp of the group it just produced
                # and the ACT engine is never starved by block detours.
                # The out-projection of chunk j-1 trickles one matmul into
                # the even slots of chunk j, the Q projection for chunk j+1
                # into the odd slots; each head's normalization is deferred
                # into the next head's stream.
                def make_oproj_ops(ctx2_src, jj):
                    """Closures: 32 po matmuls (+ copies/DMA) for chunk jj's
                    output projection from its pair-stacked ctx tiles."""
                    state = {}

                    def op(k):
                        def run():
                            t, ec, p = k // 8, (k // 4) % 2, k % 4
                            if ec == 0 and p == 0:
                                state["ob"] = obp.tile([128, D], F32,
                                                       tag="ob",
                                                       name=f"ob{jj}_{t}")
                            if p == 0:
                                state["po"] = psp.tile(
                                    [128, 512], F32, tag="sh", bufs=2,
                                    name=f"po{jj}_{t}{ec}")
                            nc.tensor.matmul(
                                state["po"][:],
                                ctx2_src[p][:, t * 128:t * 128 + 128],
                                wo_sb[:, p * D + ec * 512:
                                      p * D + ec * 512 + 512],
                                start=(p == 0), stop=(p == NDT - 1),
                            )
                            if p == NDT - 1:
                                nc.vector.tensor_copy(
                                    state["ob"][:, ec * 512:ec * 512 + 512],
                                    state["po"][:])
                                if ec == 1:
                                    nc.sync.dma_start(
                                        out[jj * 512 + t * 128:
                                            jj * 512 + t * 128 + 128, :],
                                        state["ob"][:])
                        return run
                    return [op(k) for k in range(32)]

                def make_qproj_ops(x_sb, x_aug, jj):
                    """Closures: 32 matmuls (+ copies) projecting Q chunk jj
                    into qt_sb."""
                    state = {}

                    def op(m):
                        def run():
                            dt, kt = m // 8, m % 8
                            if kt == 0:
                                state["ps"] = psp.tile(
                                    [128, 512], F32, tag="sh", bufs=2,
                                    name=f"qp{jj}_{dt}")
                            ps = state["ps"]
                            nc.tensor.matmul(
                                ps[:],
                                wq_sb[:, kt * DH + dt * 128:
                                      kt * DH + dt * 128 + 128],
                                x_sb[:, kt * 512:kt * 512 + 512],
                                start=(kt == 0),
                                stop=(kt == KT_N - 1 and not has_bias),
                            )
                            if kt == KT_N - 1:
                                if has_bias:
                                    nc.tensor.matmul(
                                        ps[:],
                                        wq_aug[0:1, dt * 128:dt * 128 + 128],
                                        x_aug[0:1, :],
                                        start=False, stop=True,
                                    )
                                nc.vector.tensor_copy(
                                    qt_sb[:, dt * S + jj * 512:
                                          dt * S + jj * 512 + 512], ps[:])
                        return run
                    return [op(m) for m in range(32)]

                norm2q = []  # (ready_at_slot, fn) stage-2 normalizations
                cur = {"gslot": 0}

                def emit_ctx(ctx_ps, h, g, at, norm_fn):
                    for b in range(G):
                        i = g * G + b
                        nc.tensor.matmul(
                            ctx_ps[:],
                            v_sb[:, i * VSTRIDE + h * (DK + 1):
                                 i * VSTRIDE + h * (DK + 1) + DK + 1],
                            at[:, b * 512:b * 512 + 512],
                            start=(g == 0 and b == 0),
                            stop=(g == NG - 1 and b == G - 1),
                        )
                    if norm_fn is not None:
                        norm_fn()

                def make_norm(ctx_ps, h, ctx2_cur):
                    """Two-stage normalization: the reciprocal (DVE) fires
                    with the head's last ctx matmul; the PE broadcast and
                    the multiply are deferred a couple of slots so the PE
                    never waits on the DVE."""
                    hp, pb = h // 2, (h % 2) * 64

                    def stage1():
                        rsr = nrm.tile([1, 512], F32R, tag="rsr",
                                       name=f"rsr{h}")
                        with nc.allow_low_precision("f32r is f32 bits"):
                            nc.vector.reciprocal(rsr[:], ctx_ps[64:65, :])

                        def stage2():
                            bc = psp.tile([64, 512], F32, tag="lg", bufs=2,
                                          name=f"bc{h}")
                            nc.tensor.matmul(
                                bc[:], ones1[:], rsr[:],
                                start=True, stop=True)
                            bc_sb = nrm.tile([64, 512], F32, tag="bcsb",
                                             name=f"bcsb{h}")
                            nc.vector.tensor_copy(bc_sb[:], bc[:])
                            if h % 2 == 0:
                                ctx2_cur[hp] = cxp.tile([128, 512], MMDT,
                                                        tag="ctx2",
                                                        name=f"ctx2_{hp}")
                            nc.vector.tensor_mul(
                                ctx2_cur[hp][pb:pb + 64, :],
                                ctx_ps[0:64, :], bc_sb[:])
                        if os.environ.get("MHA_NORM_INLINE", "0") == "1":
                            stage2()
                        else:
                            norm2q.append((cur["gslot"] + 2, stage2))
                    return stage1

                def emit_lg_exp(j, h, g):
                    hp, pb = h // 2, (h % 2) * 64
                    lg = psp.tile([128, G * 512], F32, tag="lg",
                                  bufs=2, name=f"lg{h}_{g}")
                    for b in range(G):
                        i = g * G + b
                        nc.tensor.matmul(
                            lg[:, b * 512:b * 512 + 512],
                            kt_sb[pb:pb + 64,
                                  hp * S + i * 128:hp * S + i * 128 + 128],
                            qt_sb[pb:pb + 64,
                                  hp * S + j * 512:hp * S + j * 512 + 512],
                            start=True, stop=True,
                        )
                    at = atp.tile([128, G * 512], MMDT, tag="at",
                                  name=f"at{h}_{g}")
                    if has_mask:
                        for b in range(G):
                            i = g * G + b
                            nc.scalar.activation(
                                at[:, b * 512:b * 512 + 512],
                                lg[:, b * 512:b * 512 + 512],
                                mybir.ActivationFunctionType.Exp,
                                bias=mb_sb[:, i:i + 1],
                                scale=float(SCALE),
                            )
                    else:
                        nc.scalar.activation(
                            at[:], lg[:],
                            mybir.ActivationFunctionType.Exp,
                            scale=float(SCALE),
                        )
                    return at

                # ------------------- phase A compute: K dt-major + early
                # logits/exp for heads 0-1 of chunk 0, woven one group per
                # projection block so the 2-deep lg buffer never throttles
                # the PE behind the ACT engine
                ctx2_c0 = {}
                backlog = {}  # (h, g) -> (ctx_ps, h, g, at, norm_fn)
                hemq = []
                ctx_ps_c0 = {}

                def pop_hem():
                    if hemq:
                        h, g = hemq.pop(0)
                        at = emit_lg_exp(0, h, g)
                        backlog[(h, g)] = (
                            ctx_ps_c0[h], h, g, at,
                            make_norm(ctx_ps_c0[h], h, ctx2_c0)
                            if g == NG - 1 else None)

                for dt in range(NDT):
                    for q in range(4):
                        ps = proj_psum()
                        xs, xa = xks[q]
                        for kt in range(KT_N):
                            nc.tensor.matmul(
                                ps[:],
                                w_sb[:, kt * DH + dt * 128:
                                     kt * DH + dt * 128 + 128],
                                xs[:, kt * 512:kt * 512 + 512],
                                start=(kt == 0),
                                stop=(kt == KT_N - 1 and not has_bias),
                            )
                        if has_bias:
                            nc.tensor.matmul(
                                ps[:],
                                w_aug[0:1, dt * 128:dt * 128 + 128],
                                xa[0:1, :],
                                start=False, stop=True,
                            )
                        nc.vector.tensor_copy(
                            kt_sb[:, dt * S + q * 512:dt * S + q * 512 + 512],
                            ps[:])
                        if dt == 0:
                            if q == 0:
                                qk_quarter(xq0, wq_sb, wq_aug, xq0_aug,
                                           qt_sb, 0, [0])
                                for h in (0, 1):
                                    ctx_ps_c0[h] = psp.tile(
                                        [65, 512], F32, tag="ctx", bufs=2,
                                        name=f"ctx{h}")
                            # groups 2q, 2q+1 only need this K quarter
                            if PHASE_A:
                                hemq += [(h, g)
                                         for g in (2 * q, 2 * q + 1)
                                         for h in (0, 1)]
                        pop_hem()
                    if dt > 0:
                        qk_quarter(xq0, wq_sb, wq_aug, xq0_aug, qt_sb, 0,
                                   [dt])
                    pop_hem()
                while hemq:
                    pop_hem()
                # wo: head-pair p rows -> [128, p*1024:+1024], one DMA
                nc.sync.dma_start(
                    wo_sb[:].rearrange("p (hp e) -> p hp e", e=D),
                    wo[:].rearrange("(hp p) e -> p hp e", p=128),
                )

                # ------------------- V projection, quarter-major, with the
                # deferred ctx matmuls of heads 0-1 woven in as the s-tiles
                # they need become available
                w_sb, w_aug = wv_sb, wv_aug
                for q in range(4):
                    x_sb, x_aug = xv0 if q == 0 else load_x(xv, q, f"x_v{q}")
                    for st4 in range(4):
                        st = q * 4 + st4
                        psv = proj_psum()
                        for kt in range(KT_N):
                            nc.tensor.matmul(
                                psv[:],
                                x_sb[:, kt * 512 + st4 * 128:
                                     kt * 512 + st4 * 128 + 128],
                                w_sb[:, kt * DH:(kt + 1) * DH],
                                start=(kt == 0),
                                stop=(kt == KT_N - 1 and not has_bias),
                            )
                        if has_bias:
                            nc.tensor.matmul(
                                psv[:],
                                x_aug[0:1, st4 * 128:st4 * 128 + 128],
                                w_aug[0:1, :],
                                start=False, stop=True,
                            )
                        # per-head copy into aug layout (ones cols at
                        # stride DK+1 stay untouched)
                        nc.vector.tensor_copy(
                            v_sb[:, st * VSTRIDE:st * VSTRIDE + H * (DK + 1)]
                            .rearrange("p (h c) -> p h c", c=DK + 1)
                            [:, :, 0:DK],
                            psv[:].rearrange("p (h c) -> p h c", c=DK),
                        )
                    for h in (0, 1):
                        for g in (2 * q, 2 * q + 1):
                            if (h, g) in backlog:
                                emit_ctx(*backlog.pop((h, g)))
                # flush heads 0-1's deferred normalizations before chunk 0
                # reuses their PSUM accumulator slots
                for _, fn in norm2q:
                    fn()
                norm2q.clear()

                # ------------------- steady chunks (chunk 0 from head 2)
                pend = []  # ctx matmuls, emitted 2 slots behind their exp
                ctx2_prev = None
                for j in range(NSC):
                    even_ops = (make_oproj_ops(ctx2_prev, j - 1)
                                if j > 0 else [])
                    if j + 1 < NSC:
                        xqn, xqn_aug = load_x(xq, j + 1, f"x_q{j + 1}")
                        odd_ops = make_qproj_ops(xqn, xqn_aug, j + 1)
                    else:
                        odd_ops = []
                    ctx2 = ctx2_c0 if j == 0 else {}
                    slot = 0
                    for h in range(2 if (j == 0 and PHASE_A) else 0, H):
                        ctx_ps = psp.tile([65, 512], F32, tag="ctx", bufs=2,
                                          name=f"ctx{h}")
                        for g in range(NG):
                            while norm2q and norm2q[0][0] <= cur["gslot"]:
                                norm2q.pop(0)[1]()
                            at = emit_lg_exp(j, h, g)
                            # one spread matmul: even slots advance the
                            # previous chunk's out-proj, odd slots the next
                            # chunk's Q projection
                            ops = even_ops if slot % 2 == 0 else odd_ops
                            if ops:
                                ops.pop(0)()
                            slot += 1
                            cur["gslot"] += 1
                            if len(pend) >= 2:
                                emit_ctx(*pend.pop(0))
                            pend.append((ctx_ps, h, g, at,
                                         make_norm(ctx_ps, h, ctx2)
                                         if g == NG - 1 else None))
                    for op_ in even_ops + odd_ops:  # drain leftovers
                        op_()
                    ctx2_prev = ctx2
                for p_ in pend:
                    emit_ctx(*p_)
                pend = []
                for _, fn in norm2q:
                    fn()
                norm2q.clear()
                # ---- output projection for the last q-chunk, compact
                for op_ in make_oproj_ops(ctx2_prev, NSC - 1):
                    op_()

    nc.compile()
    return nc


def _get_kernel(has_bias: bool, has_mask: bool):
    key = (has_bias, has_mask, MM_MODE, ITERS)
    if key not in _CACHE:
        _CACHE[key] = build_kernel(has_bias, has_mask)
    return _CACHE[key]


def prep_in_maps(query, key, value, mask, Wq, bq, Wk, bk, Wv, bv, Wo, bo):
    query = np.asarray(query, dtype=np.float32)
    key = np.asarray(key, dtype=np.float32)
    value = np.asarray(value, dtype=np.float32)
    mask = np.asarray(mask, dtype=np.float32)
    Wq = np.asarray(Wq, dtype=np.float32)
    Wk = np.asarray(Wk, dtype=np.float32)
    Wv = np.asarray(Wv, dtype=np.float32)
    Wo = np.asarray(Wo, dtype=np.float32)
    bq = np.asarray(bq, dtype=np.float32)
    bk = np.asarray(bk, dtype=np.float32)
    bv = np.asarray(bv, dtype=np.float32)
    bo = np.asarray(bo, dtype=np.float32)

    has_bias = bool(np.any(bq) or np.any(bk) or np.any(bv))
    has_mask = bool(np.any(mask))

    ones_row = np.ones((1, S), dtype=np.float32)
    in_maps = []
    for c in range(N_CORES):
        b, g = divmod(c, 2)
        cols = slice(g * DH, (g + 1) * DH)
        xq_c = np.ascontiguousarray(query[b].T)
        xk_c = np.ascontiguousarray(key[b].T)
        xv_c = np.ascontiguousarray(value[b].T)
        wq_c = np.ascontiguousarray(Wq[:, cols])
        wk_c = np.ascontiguousarray(Wk[:, cols])
        wv_c = np.ascontiguousarray(Wv[:, cols])
        if has_bias:
            xq_c = np.concatenate([xq_c, ones_row], axis=0)
            xk_c = np.concatenate([xk_c, ones_row], axis=0)
            xv_c = np.concatenate([xv_c, ones_row], axis=0)
            wq_c = np.concatenate([wq_c, bq[None, cols]], axis=0)
            wk_c = np.concatenate([wk_c, bk[None, cols]], axis=0)
            wv_c = np.concatenate([wv_c, bv[None, cols]], axis=0)
        m = {
            "xq": xq_c, "xk": xk_c, "xv": xv_c,
            "wq": wq_c, "wk": wk_c, "wv": wv_c,
            "wo": np.ascontiguousarray(Wo[cols, :]),
        }
        if has_mask:
            m["mb"] = np.ascontiguousarray(
                mask[b, 0, 0].reshape(NST, 128).T * np.float32(-1e9))
        in_maps.append(m)
    if MM_MODE == "bf16":
        import ml_dtypes
        for m in in_maps:
            for k in m:
                if k != "mb":
                    m[k] = m[k].astype(ml_dtypes.bfloat16)
    return has_bias, has_mask, in_maps, bo


def assemble_out(per_core_out, bo):
    out = np.empty((B, S, D), dtype=np.float32)
    for b in range(B):
        out[b] = per_core_out[2 * b] + per_core_out[2 * b + 1] + bo
    return out


def kernel(**inputs):
    has_bias, has_mask, in_maps, bo = prep_in_maps(**inputs)
    nc = _get_kernel(has_bias, has_mask)
    res = run_bass_kernel_spmd(nc, in_maps, core_ids=list(range(N_CORES)))
    return assemble_out([res.results[c]["out"] for c in range(N_CORES)], bo)


if __name__ == "__main__":
    rng = np.random.default_rng(0)
    q = rng.standard_normal((B, S, D)).astype(np.float32)
    k = rng.standard_normal((B, S, D)).astype(np.float32)
    v = rng.standard_normal((B, S, D)).astype(np.float32)
    m = np.zeros((B, 1, 1, S), dtype=np.float32)
    sc = 1.0 / np.sqrt(D)
    Wq = (rng.standard_normal((D, D)) * sc).astype(np.float32)
    Wk = (rng.standard_normal((D, D)) * sc).astype(np.float32)
    Wv = (rng.standard_normal((D, D)) * sc).astype(np.float32)
    Wo = (rng.standard_normal((D, D)) * sc).astype(np.float32)
    z = np.zeros(D, dtype=np.float32)
    o = kernel(query=q, key=k, value=v, mask=m, Wq=Wq, bq=z, Wk=Wk, bk=z,
               Wv=Wv, bv=z, Wo=Wo, bo=z)
    print("out", o.shape, o.dtype, float(np.abs(o).mean()))
